# revision 31
# baseline (speedup 1.0000x reference)
"""Luong attention (linear -> bmm -> mask -> softmax -> bmm) on 8 trn2 cores.

Reference (per batch b):
    q = h @ W.T + b                  [Tq, H]
    s = q @ x.T                      [Tq, Tk]
    s = where(mask, -inf, s)
    w = softmax(s, axis=-1)
    ctx = w @ x                      [Tq, H]

Sharding: pure data-parallel over B=16 -> 2 batches per core, no collectives.

Mask compaction: masked positions get softmax weight exactly 0, so the host
gathers only the unmasked rows of x (per batch), zero-padded to TKP (a
multiple of 64 covering the largest unmasked count across batches). Padded
columns carry an additive -1e30 mask row so exp() underflows to exactly 0,
and the padded x rows are zero so the context contribution is exactly 0.

Projection on the compacted side: score = (h@W.T + b)@x.T re-associates to
h @ (x@W).T + (x@b), so the projection matmul z = x_compact @ W contracts
over the compacted width instead of full Tq, and the bias term x@b is folded
into the host-built additive mask row for free.

dtypes: proj and score matmuls run in float16 (10-bit mantissa ~ TF32-level
for this N(0,1) data, validated rel_l2 ~3e-3 vs 2e-2 budget). fp16 gets the
fast 16-bit LDWEIGHTS path (~115ns vs 227ns for fp32), which un-bounds the
previously LDWEIGHTS-limited fp32r matmul cadence (277ns -> 216ns for
N=512 at the full 2.4GHz clock), and halves W/hT/xT DMA traffic.
Context matmul stays bf16, softmax stays fp32.

Per-core device pipeline (per batch):
    zT = W.T-tiles @ xT              fp16 matmuls over compacted width
    per 128-row chunk of Tq:
      score psum = sum_k hT_k.T @ zT_k            fp16, balanced bank groups
      masked = score + (maskrow = x@b | -1e30 pad) -> SBUF (DVE)
      negmax = -rowmax(masked); w = exp(masked - max) with accum row sums
      wT = PE-transpose(w) in bf16 via PSUM
      ctx psum = sum_j wT_j.T @ x_j               bf16
      out = ctx * (1/rowsum)                      scale on ACT, DMA (gpsimd)

Scheduling notes (measured): carry depth stays 1 — a fully dense PE
(depth-2 carry) trips the chip's P0 power downclock (2.4 -> 2.0 GHz) and
nets out ~17% slower. HAM warmup matmuls likewise measured net-negative.
"""
import numpy as np

import concourse.bacc as bacc


def _install_ntff_hook_shim():
    """The agent image's `antenv` lacks `axon_hooks`; bass_utils imports it
    for trace=True under axon. Provide it and register the ctypes hook."""
    import sys
    import types
    try:
        import antenv.axon_hooks  # noqa: F401
        return
    except ImportError:
        pass
    mod = types.ModuleType("antenv.axon_hooks")
    _state = {"hook": None}
    mod.set_axon_ntff_profile_hook = lambda h: _state.__setitem__("hook", h)
    mod.get_axon_ntff_profile_hook = lambda: _state["hook"]
    sys.modules["antenv.axon_hooks"] = mod
    try:
        import antenv
        antenv.axon_hooks = mod
    except ImportError:
        pass
    try:
        from trn_agent_boot.trn_boot import _ntff_profile_via_ctypes
        hook = _ntff_profile_via_ctypes("/opt/axon/libaxon_pjrt.so")
        if hook is not None:
            mod.set_axon_ntff_profile_hook(hook)
    except Exception:
        pass


_install_ntff_hook_shim()

import concourse.mybir as mybir  # noqa: E402
import concourse.tile as tile  # noqa: E402
from concourse.bass_utils import run_bass_kernel_spmd  # noqa: E402

F32 = mybir.dt.float32
F16 = mybir.dt.float16
BF16 = mybir.dt.bfloat16

B, TQ, TK, H = 16, 1024, 1024, 1024
NCORES = 8
BPC = B // NCORES          # batches per core
P = 128
KT = H // P                # 8 k-tiles of the contraction dims
CH = TQ // P               # 8 q-row chunks per batch
NH = TQ // 512             # q-phase free-dim halves

_MASK_NEG = np.float32(-1e30)


def _score_groups(tkp):
    """Bank-packed group sizes: <=512 per PSUM bank, balanced (fp16 LD
    ~115ns hides under any balanced group's stream)."""
    ng = (tkp + 511) // 512
    base = tkp // ng
    gs = [base // 64 * 64] * ng
    rem = tkp - sum(gs)
    i = 0
    while rem > 0:
        gs[i] += 64
        rem -= 64
        i = (i + 1) % ng
    return gs


def _j_tiles(tkp):
    """128-row tiles covering tkp, last one possibly 64."""
    out = []
    rem = tkp
    while rem > 0:
        js = min(P, rem)
        out.append(js)
        rem -= js
    return out


def _build_nc(tkp0, tkp1):
    tkpm = max(tkp0, tkp1)     # DRAM tensors sized to the wider slot
    jtm = len(_j_tiles(tkpm))
    nc = bacc.Bacc("TRN2", target_bir_lowering=False)
    # Wm: [m, 128, H] m-major column tiles of W.T, pre-packed [p, k, c]
    Wm_d = nc.dram_tensor("Wm", [KT, P, H], F16, kind="ExternalInput")
    # hT tiled [b, n, k, 128, 512]
    hT_d = nc.dram_tensor("hT", [BPC, NH, KT, P, 512], F16, kind="ExternalInput")
    xT_d = nc.dram_tensor("xT", [BPC, H, tkpm], F16, kind="ExternalInput")
    xn_d = nc.dram_tensor("xn", [BPC, tkpm, H], BF16, kind="ExternalInput")
    am_d = nc.dram_tensor("amask", [BPC, tkpm], F32, kind="ExternalInput")
    ctx_d = nc.dram_tensor("ctx", [BPC, TQ, H], F32, kind="ExternalOutput")

    with tile.TileContext(nc) as tc:
        with (
            tc.tile_pool(name="consts", bufs=1) as consts,
            tc.tile_pool(name="hTp", bufs=2) as hT_pool,
            tc.tile_pool(name="qTp", bufs=1) as qT_pool,
            tc.tile_pool(name="xTp", bufs=1) as xT_pool,
            tc.tile_pool(name="xnp", bufs=1) as xn_pool,
            tc.tile_pool(name="chk", bufs=2) as chk,
            # b0 streams both output halves through gpsimd (~5.8us of
            # transfer per ~4.3us chunk), so the out staging ring needs
            # 3 slots to absorb the transfer backlog without stalling
            # the scale pipeline.
            tc.tile_pool(name="outp", bufs=3) as out_pool,
            tc.tile_pool(name="stat", bufs=2) as stat,
            tc.tile_pool(name="ps_s", bufs=2, space="PSUM") as ps_s,
            tc.tile_pool(name="ps_w", bufs=1, space="PSUM") as ps_w,
            tc.tile_pool(name="ps_c", bufs=1, space="PSUM") as ps_c,
        ):
            from concourse.masks import make_identity
            ident = consts.tile([P, P], BF16, tag="ident")
            make_identity(nc, ident)
            w_sb = consts.tile([P, KT, KT, P], F16, tag="W")  # [p, m, k, c]

            def dma_w(m):
                # W stays whole on scalar/HWDGE: gpsimd's SWDGE path moves
                # this strided [p,k,c] pattern ~2x slower (measured), so
                # splitting W across it delays the projection instead.
                nc.scalar.dma_start(
                    w_sb[:, m],
                    Wm_d[m].rearrange("p (k c) -> p k c", k=KT),
                )

            for b in range(BPC):
                tkp = (tkp0, tkp1)[b]   # per-slot compacted width
                jts = _j_tiles(tkp)
                jt = len(jts)
                # ---- input DMAs, first-needed-first: the projection
                # z = x@W needs xT + W; hT is only the score stationary ----
                xT_sb = xT_pool.tile([P, KT, tkp], F16, tag="xT")
                if b == 0:
                    dma_w(0)
                for k in range(KT):
                    nc.sync.dma_start(
                        xT_sb[:, k], xT_d[b, k * P:(k + 1) * P, 0:tkp])
                if b == 0:
                    for m in range(1, KT):
                        dma_w(m)
                hT_sb = hT_pool.tile([P, KT, TQ], F16, tag="hT")
                for n in range(NH):
                    for k in range(KT):
                        nc.sync.dma_start(
                            hT_sb[:, k, n * 512:(n + 1) * 512], hT_d[b, n, k]
                        )
                mask_sb = stat.tile([P, tkp], F32, tag="maskb")
                nc.scalar.dma_start(
                    mask_sb, am_d[b:b + 1, 0:tkp].partition_broadcast(P)
                )
                xn_sb = xn_pool.tile([P, jt, H], BF16, tag="xn")
                for j, js in enumerate(jts):
                    nc.scalar.dma_start(
                        xn_sb[0:js, j], xn_d[b, j * P:j * P + js, :])

                # ---- projection over the COMPACTED width:
                # zT[m][i, s] = sum_k Wn_k,m.T @ xT_k  (z = x @ W) ----
                zT_sb = qT_pool.tile([P, KT, tkp], F16, tag="qT")
                for m in range(KT):
                    gs = 0
                    for gn in _score_groups(tkp):
                        zp = ps_s.tile([P, 512], F32, tag="sp")
                        for k in range(KT):
                            nc.tensor.matmul(
                                zp[:, 0:gn],
                                w_sb[:, m, k],
                                xT_sb[:, k, gs:gs + gn],
                                start=(k == 0),
                                stop=(k == KT - 1),
                            )
                        nc.vector.tensor_copy(
                            zT_sb[:, m, gs:gs + gn], zp[:, 0:gn])
                        gs += gn

                # ---- chunk pipeline over Tq rows, carry depth 1 ----
                carry = None
                for c in range(CH + 1):
                    if carry is not None:
                        pc, w_t, rsum_t = carry
                        wTp = ps_w.tile([P, jtm * P], BF16, tag="wt")
                        for j, js in enumerate(jts):
                            nc.tensor.transpose(
                                wTp[0:js, j * P:(j + 1) * P],
                                w_t[:, j * P:j * P + js],
                                ident,
                            )
                        wT_sb = chk.tile([P, jtm * P], BF16, tag="wT")
                        nc.vector.tensor_copy(
                            wT_sb[:, 0:jt * P], wTp[:, 0:jt * P])

                    if c < CH:
                        groups = _score_groups(tkp)
                        sp = ps_s.tile([P, len(groups), 512], F32, tag="sp")
                        gs = 0
                        for gi, gn in enumerate(groups):
                            for k in range(KT):
                                nc.tensor.matmul(
                                    sp[:, gi, 0:gn],
                                    hT_sb[:, k, c * P:(c + 1) * P],
                                    zT_sb[:, k, gs:gs + gn],
                                    start=(k == 0),
                                    stop=(k == KT - 1),
                                )
                            gs += gn
                        sc_sb = chk.tile([P, tkp], F32, tag="sc")
                        gs = 0
                        for gi, gn in enumerate(groups):
                            nc.vector.tensor_add(
                                sc_sb[:, gs:gs + gn],
                                sp[:, gi, 0:gn],
                                mask_sb[:, gs:gs + gn],
                            )
                            gs += gn
                        negmax = stat.tile([P, 1], F32, tag="negmax")
                        nc.vector.reduce_max(
                            negmax, sc_sb, axis=mybir.AxisListType.X, negate=True
                        )
                        w_new = chk.tile([P, tkp], BF16, tag="w")
                        ssum = stat.tile([P, 1], F32, tag="ssum")
                        nc.scalar.activation(
                            w_new, sc_sb, mybir.ActivationFunctionType.Exp,
                            bias=negmax, scale=1.0, accum_out=ssum,
                        )
                        rsum_new = stat.tile([P, 1], F32, tag="rsum")
                        nc.vector.reciprocal(rsum_new, ssum)

                    if carry is not None:
                        pc, w_t, rsum_t = carry
                        cxp = ps_c.tile([P, H], F32, tag="cx")
                        for n in range(H // 512):
                            for j, js in enumerate(jts):
                                nc.tensor.matmul(
                                    cxp[:, n * 512:(n + 1) * 512],
                                    wT_sb[0:js, j * P:(j + 1) * P],
                                    xn_sb[0:js, j, n * 512:(n + 1) * 512],
                                    start=(j == 0),
                                    stop=(j == jt - 1),
                                )
                        outc = out_pool.tile([P, H], F32, tag="outc")
                        for n in range(H // 512):
                            nsl = slice(n * 512, (n + 1) * 512)
                            nc.scalar.activation(
                                outc[:, nsl], cxp[:, nsl],
                                mybir.ActivationFunctionType.Copy,
                                scale=rsum_t,
                            )
                            # b1's n=1 outputs ride sync, which is idle once
                            # b1's inputs have landed (~67us): halves gpsimd
                            # queue pressure in the second half and removes
                            # one serial 2.9us transfer from the final-chunk
                            # drain tail. (b0's outputs must stay off sync —
                            # it still carries b1's input stream then.)
                            oeng = nc.sync if (b == 1 and n == 1) else nc.gpsimd
                            oeng.dma_start(
                                ctx_d[b, pc * P:(pc + 1) * P, nsl],
                                outc[:, nsl],
                            )

                    carry = (c, w_new, rsum_new) if c < CH else None
    return nc


_CACHE = {}


def _get_nc(tkp0, tkp1):
    key = (tkp0, tkp1)
    if key not in _CACHE:
        nc = _build_nc(tkp0, tkp1)
        nc.compile()
        _CACHE[key] = nc
    return _CACHE[key]


def kernel(h_t_dec, x_enc, mask, W, b, _trace=False, _trace_kwargs=None):
    import ml_dtypes

    h_t_dec = np.ascontiguousarray(h_t_dec, dtype=np.float32)
    x_enc = np.ascontiguousarray(x_enc, dtype=np.float32)
    mask = np.asarray(mask).astype(bool)
    W = np.ascontiguousarray(W, dtype=np.float32)
    b = np.ascontiguousarray(b, dtype=np.float32)

    # Wn[m, p, k, c] = W[k*128 + p, m*128 + c] (W natural: kxm for z = x@W)
    Wm = np.ascontiguousarray(
        W.reshape(KT, P, KT, P).transpose(2, 1, 0, 3).reshape(KT, P, H)
    ).astype(np.float16)

    keep = [np.nonzero(~mask[bi])[0] for bi in range(B)]

    def pad64(n):
        return min(TK, max(P, ((n + 63) // 64) * 64))

    # Load-balance: sort batches by unmasked count; slot 0 takes the 8
    # smallest, slot 1 the 8 largest, so each slot's compiled width covers
    # only its own worst case instead of the global max.
    order = np.argsort([len(k) for k in keep], kind="stable")
    slot_batches = [order[:NCORES], order[NCORES:]]        # [slot][core]
    tkp0 = pad64(max(len(keep[g]) for g in slot_batches[0]))
    tkp1 = pad64(max(len(keep[g]) for g in slot_batches[1]))
    tkpm = max(tkp0, tkp1)

    # compacted x at the max width: unmasked rows first, zero rows beyond
    xc = np.zeros((B, tkpm, H), dtype=np.float32)
    amask_full = np.full((B, tkpm), _MASK_NEG, dtype=np.float32)
    for bi in range(B):
        nk = len(keep[bi])
        xc[bi, :nk] = x_enc[bi, keep[bi]]
        # score = h @ (x@W).T + (x@b): fold the bias term into the mask row
        amask_full[bi, :nk] = (
            xc[bi, :nk].astype(np.float64) @ b.astype(np.float64)
        ).astype(np.float32)

    in_maps = []
    for core in range(NCORES):
        gb = [slot_batches[0][core], slot_batches[1][core]]
        hT = h_t_dec[gb].transpose(0, 2, 1)              # [b, H, Tq]
        hT_t = np.ascontiguousarray(
            hT.reshape(BPC, KT, P, NH, 512).transpose(0, 3, 1, 2, 4)
        ).astype(np.float16)
        xT = np.ascontiguousarray(xc[gb].transpose(0, 2, 1)).astype(np.float16)
        xn = np.ascontiguousarray(xc[gb]).astype(ml_dtypes.bfloat16)
        in_maps.append({
            "hT": hT_t,
            "xT": xT,
            "xn": xn,
            "Wm": Wm,
            "amask": np.ascontiguousarray(amask_full[gb]),
        })

    nc = _get_nc(tkp0, tkp1)
    res = run_bass_kernel_spmd(
        nc, in_maps, core_ids=list(range(NCORES)),
        trace=_trace, trace_kwargs=_trace_kwargs or {},
    )
    out = np.empty((B, TQ, H), dtype=np.float32)
    for core in range(NCORES):
        out[slot_batches[0][core]] = res.results[core]["ctx"][0]
        out[slot_batches[1][core]] = res.results[core]["ctx"][1]
    if _trace:
        return out, res
    return out


# revision 33
# speedup vs baseline: 1.0208x; 1.0208x over previous
"""Luong attention (linear -> bmm -> mask -> softmax -> bmm) on 8 trn2 cores.

Reference (per batch b):
    q = h @ W.T + b                  [Tq, H]
    s = q @ x.T                      [Tq, Tk]
    s = where(mask, -inf, s)
    w = softmax(s, axis=-1)
    ctx = w @ x                      [Tq, H]

Sharding: pure data-parallel over B=16 -> 2 batches per core, no collectives.

Mask compaction: masked positions get softmax weight exactly 0, so the host
gathers only the unmasked rows of x (per batch), zero-padded to TKP (a
multiple of 64 covering the largest unmasked count across batches). Padded
columns carry an additive -1e30 mask row so exp() underflows to exactly 0,
and the padded x rows are zero so the context contribution is exactly 0.

Projection on the compacted side: score = (h@W.T + b)@x.T re-associates to
h @ (x@W).T + (x@b), so the projection matmul z = x_compact @ W contracts
over the compacted width instead of full Tq, and the bias term x@b is folded
into the host-built additive mask row for free.

dtypes: proj and score matmuls run in float16 (10-bit mantissa ~ TF32-level
for this N(0,1) data, validated rel_l2 ~3e-3 vs 2e-2 budget). fp16 gets the
fast 16-bit LDWEIGHTS path (~115ns vs 227ns for fp32), which un-bounds the
previously LDWEIGHTS-limited fp32r matmul cadence (277ns -> 216ns for
N=512 at the full 2.4GHz clock), and halves W/hT/xT DMA traffic.
Context matmul stays bf16, softmax stays fp32.

Per-core device pipeline (per batch):
    zT = W.T-tiles @ xT              fp16 matmuls over compacted width
    per 128-row chunk of Tq:
      score psum = sum_k hT_k.T @ zT_k            fp16, balanced bank groups
      masked = score + (maskrow = x@b | -1e30 pad) -> SBUF (DVE)
      negmax = -rowmax(masked); w = exp(masked - max) with accum row sums
      wT = PE-transpose(w) in bf16 via PSUM
      ctx psum = sum_j wT_j.T @ x_j               bf16
      out = ctx * (1/rowsum)                      scale on ACT, DMA (gpsimd)

Scheduling notes (measured): carry depth stays 1 — a fully dense PE
(depth-2 carry) trips the chip's P0 power downclock (2.4 -> 2.0 GHz) and
nets out ~17% slower. HAM warmup matmuls likewise measured net-negative.
"""
import numpy as np

import concourse.bacc as bacc


def _install_ntff_hook_shim():
    """The agent image's `antenv` lacks `axon_hooks`; bass_utils imports it
    for trace=True under axon. Provide it and register the ctypes hook."""
    import sys
    import types
    try:
        import antenv.axon_hooks  # noqa: F401
        return
    except ImportError:
        pass
    mod = types.ModuleType("antenv.axon_hooks")
    _state = {"hook": None}
    mod.set_axon_ntff_profile_hook = lambda h: _state.__setitem__("hook", h)
    mod.get_axon_ntff_profile_hook = lambda: _state["hook"]
    sys.modules["antenv.axon_hooks"] = mod
    try:
        import antenv
        antenv.axon_hooks = mod
    except ImportError:
        pass
    try:
        from trn_agent_boot.trn_boot import _ntff_profile_via_ctypes
        hook = _ntff_profile_via_ctypes("/opt/axon/libaxon_pjrt.so")
        if hook is not None:
            mod.set_axon_ntff_profile_hook(hook)
    except Exception:
        pass


_install_ntff_hook_shim()

import concourse.mybir as mybir  # noqa: E402
import concourse.tile as tile  # noqa: E402
from concourse.bass_utils import run_bass_kernel_spmd  # noqa: E402

F32 = mybir.dt.float32
F16 = mybir.dt.float16
BF16 = mybir.dt.bfloat16

B, TQ, TK, H = 16, 1024, 1024, 1024
NCORES = 8
BPC = B // NCORES          # batches per core
P = 128
KT = H // P                # 8 k-tiles of the contraction dims
CH = TQ // P               # 8 q-row chunks per batch
NH = TQ // 512             # q-phase free-dim halves

_MASK_NEG = np.float32(-1e30)


def _score_groups(tkp):
    """Bank-packed group sizes: <=512 per PSUM bank, balanced (fp16 LD
    ~115ns hides under any balanced group's stream)."""
    ng = (tkp + 511) // 512
    base = tkp // ng
    gs = [base // 64 * 64] * ng
    rem = tkp - sum(gs)
    i = 0
    while rem > 0:
        gs[i] += 64
        rem -= 64
        i = (i + 1) % ng
    return gs


def _j_tiles(tkp):
    """128-row tiles covering tkp, last one possibly 64."""
    out = []
    rem = tkp
    while rem > 0:
        js = min(P, rem)
        out.append(js)
        rem -= js
    return out


def _build_nc(tkp0, tkp1):
    tkpm = max(tkp0, tkp1)     # DRAM tensors sized to the wider slot
    jtm = len(_j_tiles(tkpm))
    nc = bacc.Bacc("TRN2", target_bir_lowering=False)
    # Wm: [m, 128, H] m-major column tiles of W.T, pre-packed [p, k, c]
    Wm_d = nc.dram_tensor("Wm", [KT, P, H], F16, kind="ExternalInput")
    # hT tiled [b, n, k, 128, 512]
    hT_d = nc.dram_tensor("hT", [BPC, NH, KT, P, 512], F16, kind="ExternalInput")
    xT_d = nc.dram_tensor("xT", [BPC, H, tkpm], F16, kind="ExternalInput")
    xn_d = nc.dram_tensor("xn", [BPC, tkpm, H], BF16, kind="ExternalInput")
    am_d = nc.dram_tensor("amask", [BPC, tkpm], F32, kind="ExternalInput")
    ctx_d = nc.dram_tensor("ctx", [BPC, TQ, H], F32, kind="ExternalOutput")

    with tile.TileContext(nc) as tc:
        with (
            tc.tile_pool(name="consts", bufs=1) as consts,
            tc.tile_pool(name="hTp", bufs=2) as hT_pool,
            tc.tile_pool(name="qTp", bufs=1) as qT_pool,
            tc.tile_pool(name="xTp", bufs=1) as xT_pool,
            tc.tile_pool(name="xnp", bufs=1) as xn_pool,
            tc.tile_pool(name="chk", bufs=2) as chk,
            # b0 streams both output halves through gpsimd (~5.8us of
            # transfer per ~4.3us chunk), so the out staging ring needs
            # 3 slots to absorb the transfer backlog without stalling
            # the scale pipeline.
            tc.tile_pool(name="outp", bufs=3) as out_pool,
            tc.tile_pool(name="stat", bufs=2) as stat,
            tc.tile_pool(name="ps_s", bufs=2, space="PSUM") as ps_s,
            tc.tile_pool(name="ps_w", bufs=1, space="PSUM") as ps_w,
            tc.tile_pool(name="ps_c", bufs=1, space="PSUM") as ps_c,
        ):
            from concourse.masks import make_identity
            ident = consts.tile([P, P], BF16, tag="ident")
            make_identity(nc, ident)
            w_sb = consts.tile([P, KT, KT, P], F16, tag="W")  # [p, m, k, c]

            def dma_w(m):
                # W stays whole on scalar/HWDGE: gpsimd's SWDGE path moves
                # this strided [p,k,c] pattern ~2x slower (measured), so
                # splitting W across it delays the projection instead.
                nc.scalar.dma_start(
                    w_sb[:, m],
                    Wm_d[m].rearrange("p (k c) -> p k c", k=KT),
                )

            for b in range(BPC):
                tkp = (tkp0, tkp1)[b]   # per-slot compacted width
                jts = _j_tiles(tkp)
                jt = len(jts)
                # ---- input DMAs, first-needed-first: the projection
                # z = x@W needs xT + W; hT is only the score stationary ----
                xT_sb = xT_pool.tile([P, KT, tkp], F16, tag="xT")
                if b == 0:
                    dma_w(0)
                for k in range(KT):
                    nc.sync.dma_start(
                        xT_sb[:, k], xT_d[b, k * P:(k + 1) * P, 0:tkp])
                if b == 0:
                    for m in range(1, KT):
                        dma_w(m)
                hT_sb = hT_pool.tile([P, KT, TQ], F16, tag="hT")
                for n in range(NH):
                    for k in range(KT):
                        nc.sync.dma_start(
                            hT_sb[:, k, n * 512:(n + 1) * 512], hT_d[b, n, k]
                        )
                mask_sb = stat.tile([P, tkp], F32, tag="maskb")
                nc.scalar.dma_start(
                    mask_sb, am_d[b:b + 1, 0:tkp].partition_broadcast(P)
                )
                xn_sb = xn_pool.tile([P, jt, H], BF16, tag="xn")
                for j, js in enumerate(jts):
                    nc.scalar.dma_start(
                        xn_sb[0:js, j], xn_d[b, j * P:j * P + js, :])

                # ---- projection over the COMPACTED width:
                # zT[m][i, s] = sum_k Wn_k,m.T @ xT_k  (z = x @ W) ----
                zT_sb = qT_pool.tile([P, KT, tkp], F16, tag="qT")
                for m in range(KT):
                    gs = 0
                    for gn in _score_groups(tkp):
                        zp = ps_s.tile([P, 512], F32, tag="sp")
                        for k in range(KT):
                            nc.tensor.matmul(
                                zp[:, 0:gn],
                                w_sb[:, m, k],
                                xT_sb[:, k, gs:gs + gn],
                                start=(k == 0),
                                stop=(k == KT - 1),
                            )
                        nc.vector.tensor_copy(
                            zT_sb[:, m, gs:gs + gn], zp[:, 0:gn])
                        gs += gn

                # ---- chunk pipeline over Tq rows, carry depth 1 ----
                carry = None
                for c in range(CH + 1):
                    if carry is not None:
                        pc, w_t, rsum_t = carry
                        wTp = ps_w.tile([P, jtm * P], BF16, tag="wt")
                        for j, js in enumerate(jts):
                            nc.tensor.transpose(
                                wTp[0:js, j * P:(j + 1) * P],
                                w_t[:, j * P:j * P + js],
                                ident,
                            )
                        wT_sb = chk.tile([P, jtm * P], BF16, tag="wT")
                        nc.vector.tensor_copy(
                            wT_sb[:, 0:jt * P], wTp[:, 0:jt * P])

                    if c < CH:
                        groups = _score_groups(tkp)
                        sp = ps_s.tile([P, len(groups), 512], F32, tag="sp")
                        gs = 0
                        for gi, gn in enumerate(groups):
                            for k in range(KT):
                                nc.tensor.matmul(
                                    sp[:, gi, 0:gn],
                                    hT_sb[:, k, c * P:(c + 1) * P],
                                    zT_sb[:, k, gs:gs + gn],
                                    start=(k == 0),
                                    stop=(k == KT - 1),
                                )
                            gs += gn
                        sc_sb = chk.tile([P, tkp], F32, tag="sc")
                        gs = 0
                        for gi, gn in enumerate(groups):
                            nc.vector.tensor_add(
                                sc_sb[:, gs:gs + gn],
                                sp[:, gi, 0:gn],
                                mask_sb[:, gs:gs + gn],
                            )
                            gs += gn
                        negmax = stat.tile([P, 1], F32, tag="negmax")
                        nc.vector.reduce_max(
                            negmax, sc_sb, axis=mybir.AxisListType.X, negate=True
                        )
                        w_new = chk.tile([P, tkp], BF16, tag="w")
                        ssum = stat.tile([P, 1], F32, tag="ssum")
                        nc.scalar.activation(
                            w_new, sc_sb, mybir.ActivationFunctionType.Exp,
                            bias=negmax, scale=1.0, accum_out=ssum,
                        )
                        rsum_new = stat.tile([P, 1], F32, tag="rsum")
                        nc.vector.reciprocal(rsum_new, ssum)

                    if carry is not None:
                        pc, w_t, rsum_t = carry
                        cxp = ps_c.tile([P, H], F32, tag="cx")
                        for n in range(H // 512):
                            for j, js in enumerate(jts):
                                nc.tensor.matmul(
                                    cxp[:, n * 512:(n + 1) * 512],
                                    wT_sb[0:js, j * P:(j + 1) * P],
                                    xn_sb[0:js, j, n * 512:(n + 1) * 512],
                                    start=(j == 0),
                                    stop=(j == jt - 1),
                                )
                        outc = out_pool.tile([P, H], F32, tag="outc")
                        for n in range(H // 512):
                            nsl = slice(n * 512, (n + 1) * 512)
                            nc.scalar.activation(
                                outc[:, nsl], cxp[:, nsl],
                                mybir.ActivationFunctionType.Copy,
                                scale=rsum_t,
                            )
                            # b1's n=1 outputs ride sync, which is idle once
                            # b1's inputs have landed (~67us): halves gpsimd
                            # queue pressure in the second half and removes
                            # one serial 2.9us transfer from the final-chunk
                            # drain tail. (b0's outputs must stay off sync —
                            # it still carries b1's input stream then.)
                            oeng = nc.sync if (b == 1 and n == 1) else nc.gpsimd
                            oeng.dma_start(
                                ctx_d[b, pc * P:(pc + 1) * P, nsl],
                                outc[:, nsl],
                            )

                    carry = (c, w_new, rsum_new) if c < CH else None
    return nc


_CACHE = {}


def _get_nc(tkp0, tkp1):
    key = (tkp0, tkp1)
    if key not in _CACHE:
        nc = _build_nc(tkp0, tkp1)
        nc.compile()
        _CACHE[key] = nc
    return _CACHE[key]


def kernel(h_t_dec, x_enc, mask, W, b, _trace=False, _trace_kwargs=None):
    import ml_dtypes

    h_t_dec = np.ascontiguousarray(h_t_dec, dtype=np.float32)
    x_enc = np.ascontiguousarray(x_enc, dtype=np.float32)
    mask = np.asarray(mask).astype(bool)
    W = np.ascontiguousarray(W, dtype=np.float32)
    b = np.ascontiguousarray(b, dtype=np.float32)

    # Wn[m, p, k, c] = W[k*128 + p, m*128 + c] (W natural: kxm for z = x@W)
    Wm = np.ascontiguousarray(
        W.reshape(KT, P, KT, P).transpose(2, 1, 0, 3).reshape(KT, P, H)
    ).astype(np.float16)

    keep = [np.nonzero(~mask[bi])[0] for bi in range(B)]

    def pad64(n):
        return min(TK, max(P, ((n + 63) // 64) * 64))

    # Load-balance: sort batches by unmasked count; slot 0 takes the 8
    # smallest, slot 1 the 8 largest, so each slot's compiled width covers
    # only its own worst case instead of the global max.
    order = np.argsort([len(k) for k in keep], kind="stable")
    slot_batches = [order[:NCORES], order[NCORES:]]        # [slot][core]
    tkp0 = pad64(max(len(keep[g]) for g in slot_batches[0]))
    tkp1 = pad64(max(len(keep[g]) for g in slot_batches[1]))
    tkpm = max(tkp0, tkp1)

    # compacted x at the max width: unmasked rows first, zero rows beyond
    xc = np.zeros((B, tkpm, H), dtype=np.float32)
    amask_full = np.full((B, tkpm), _MASK_NEG, dtype=np.float32)
    for bi in range(B):
        nk = len(keep[bi])
        xc[bi, :nk] = x_enc[bi, keep[bi]]
        # score = h @ (x@W).T + (x@b): fold the bias term into the mask row
        amask_full[bi, :nk] = (
            xc[bi, :nk].astype(np.float64) @ b.astype(np.float64)
        ).astype(np.float32)

    in_maps = []
    for core in range(NCORES):
        gb = [slot_batches[0][core], slot_batches[1][core]]
        hT = h_t_dec[gb].transpose(0, 2, 1)              # [b, H, Tq]
        hT_t = np.ascontiguousarray(
            hT.reshape(BPC, KT, P, NH, 512).transpose(0, 3, 1, 2, 4)
        ).astype(np.float16)
        xT = np.ascontiguousarray(xc[gb].transpose(0, 2, 1)).astype(np.float16)
        xn = np.ascontiguousarray(xc[gb]).astype(ml_dtypes.bfloat16)
        in_maps.append({
            "hT": hT_t,
            "xT": xT,
            "xn": xn,
            "Wm": Wm,
            "amask": np.ascontiguousarray(amask_full[gb]),
        })

    nc = _get_nc(tkp0, tkp1)
    res = run_bass_kernel_spmd(
        nc, in_maps, core_ids=list(range(NCORES)),
        trace=_trace, trace_kwargs=_trace_kwargs or {},
    )
    out = np.empty((B, TQ, H), dtype=np.float32)
    for core in range(NCORES):
        out[slot_batches[0][core]] = res.results[core]["ctx"][0]
        out[slot_batches[1][core]] = res.results[core]["ctx"][1]
    if _trace:
        return out, res
    return out
